# revision 18
# baseline (speedup 1.0000x reference)
"""ConvLSTM block Trainium2 kernel (8 NeuronCores).

Sharding: 8 cores = 4 batches x 2 H-halves. Bottom-half cores process their
slab vertically flipped (with kh-flipped conv kernels) so one SPMD program
serves all cores.

Cross-core traffic is ELIMINATED by redundant computation: each core
computes 48 rows/step for steps 1-7, 40 rows/step for steps 8-15, and 32
rows at step 16. The valid region shrinks by one row per step from 48, so
rows 0..31 stay valid through all 16 steps with no exchange at all (a
measured 2-rank AllReduce costs ~37us and stalls the PE - redundant matmuls
are cheaper).

Per step, chunks of 8 output rows are processed in PAIRS sharing a 2-bank
[128,1024] PSUM tile per m-half, so the epilogue runs [*,1024]-wide ops:
ACT: merged hard-sigmoid(i,f) on 128 partitions, tanh g, hard-sigmoid o,
tanh c; DVE: one stt for both gate products, h-store; GpSimd: c-combine add,
h duplicate copy, fused BN (tensor_scalar mult+add).
"""
import os
import numpy as np

T, H2, W2, F, CIN = 16, 64, 64, 64, 32
WP3 = 68
R = 32            # rows owned per core
RC = 48           # max rows computed per step (redundant tail shrinks)
NQ = 8            # rows per chunk (512 px = 1 PSUM bank per 128ch tile)
WP, XW = 66, 130
SLAB = 2 * RC + 1
MM_DT = os.environ.get("CONV_LSTM_MM_DT", "bf16")  # bf16 | fp32 | fp32r

_CACHE = {}


def _storage_np_dtype():
    import ml_dtypes
    return ml_dtypes.bfloat16 if MM_DT == "bf16" else np.float32


def _prep_core_inputs(x, W, U, b, gamma, beta, moving_mean, moving_var,
                      bidx, half):
    sdt = _storage_np_dtype()
    flip = (half == 1)

    # x slab [T, CIN, SLAB, XW]; XLA SAME (stride2,k3,even) pads bottom/right
    # only: out row r reads input rows 2r..2r+2 (row/col 128 = zero pad).
    xs = np.zeros((T, CIN, SLAB, XW), np.float32)
    xc = np.ascontiguousarray(x[bidx].transpose(0, 3, 1, 2))  # (T,CIN,128,128)
    if not flip:
        xs[:, :, 0:SLAB, 0:128] = xc[:, :, 0:SLAB, :]
    else:
        # slab[s] = x_global[128 - s]; s=0 is the zero pad row
        xs[:, :, 1:SLAB, 0:128] = xc[:, :, 128 - SLAB + 1:][:, :, ::-1, :]

    Wk = W[::-1].copy() if flip else W
    Uk = U[::-1].copy() if flip else U

    w3 = np.zeros((96, 768), np.float32)
    ua = np.zeros((128, 768), np.float32)
    ub = np.zeros((128, 256), np.float32)
    for di in range(3):
        for m in range(2):
            g = di * 2 + m
            cols = slice(g * 128, (g + 1) * 128)
            mc = slice(m * 128, (m + 1) * 128)
            for j in range(3):
                w3[32*j:32*j+32, cols] = Wk[j, di, :, mc]
            ua[0:64, cols] = Uk[0, di, :, mc]
            ua[64:128, cols] = Uk[1, di, :, mc]
    for m in range(2):
        mc = slice(m * 128, (m + 1) * 128)
        # uc: (kh2, dj=-1) and (kh2, dj=+1) stacked along K
        ub[0:64, m*128:(m+1)*128] = Uk[2, 0, :, mc]
        ub[64:128, m*128:(m+1)*128] = Uk[2, 2, :, mc]
    ubs = np.zeros((64, 256), np.float32)
    for m in range(2):
        mc = slice(m * 128, (m + 1) * 128)
        ubs[0:64, m*128:(m+1)*128] = Uk[2, 1, :, mc]

    eps = 1e-3
    scale = (gamma / np.sqrt(moving_var + eps)).astype(np.float32)
    beta2 = (beta - moving_mean * scale).astype(np.float32)
    vecs = np.zeros((128, 8), np.float32)
    vecs[:, 0] = 0.2 * b[0:128] + 0.5
    vecs[0:64, 1] = b[128:192]
    vecs[64:128, 2] = 0.2 * b[192:256] + 0.5
    vecs[0:64, 3] = scale
    vecs[0:64, 4] = beta2
    return {
        "xs": np.ascontiguousarray(xs.astype(sdt)),
        "w3": np.ascontiguousarray(w3.astype(sdt)),
        "ua": np.ascontiguousarray(ua.astype(sdt)),
        "ub": np.ascontiguousarray(ub.astype(sdt)),
        "ubs": np.ascontiguousarray(ubs.astype(sdt)),
        "vecs": vecs,
    }


def _patch_tile_drain():
    """This walrus build encodes at most ONE sync wait per CTRL instruction;
    split the Tile exit drain's waits across SP nops."""
    import bass_rust
    import concourse.tile as tile
    from concourse.vector_clock import ScopedClock
    if getattr(tile.TileContext, "_drain_patched", False):
        return

    def patched(self, tick_clock, wait_clock):
        drain_inst = self.nc.sync.drain()
        wait_clock.add_sem_waits(
            drain_inst.ins, ScopedClock({None: tick_clock.global_clock}))
        si = drain_inst.ins.sync_info
        waits = list(si.on_wait) if si is not None else []
        if len(waits) > 1:
            si.on_wait = waits[:1]
            for w in waits[1:]:
                nop = self.nc.sync.nop()
                nsi = nop.ins.sync_info
                if nsi is None:
                    nop.ins.sync_info = bass_rust.SyncInfo(
                        on_wait=[w], on_update=[])
                else:
                    nsi.on_wait = [w]
        self.nc.all_engine_barrier()
        assert self.sems is not None
        popped = self.nc._tile_sem_poison_stack.pop()
        assert popped is self._sem_poison
        self.nc.clear_and_free_semaphores(list(self.sems.allocated().values()))
        self.nc.all_engine_barrier()

    tile.TileContext._drain_and_barrier = patched
    tile.TileContext._drain_patched = True


def _split_multi_waits(nc, mybir):
    """This walrus build encodes at most one sync wait per instruction;
    move excess waits onto single-wait nops inserted just before."""
    ctr = 0
    for bb in nc.main_func.blocks:
        insts = bb.instructions
        out = []
        changed = False
        for inst in insts:
            si = inst.sync_info
            waits = list(si.on_wait) if si is not None else []
            if len(waits) > 1:
                changed = True
                for w in waits[:-1]:
                    ctr += 1
                    out.append(mybir.InstNoOp(
                        name=f"wsplit-{ctr}",
                        engine=inst.engine,
                        sync_info=mybir.SyncInfo(on_wait=[w], on_update=[]),
                        bass_nofuse=True))
                si.on_wait = [waits[-1]]
            out.append(inst)
        if changed:
            bb.instructions = out
    return nc


def _build_nc():
    import concourse.bass as bass
    import concourse.mybir as mybir
    import concourse.tile as tile
    _patch_tile_drain()
    dt = mybir.dt
    sdt = dt.bfloat16 if MM_DT == "bf16" else dt.float32
    AF = mybir.ActivationFunctionType
    ALU = mybir.AluOpType

    def mm_ap(ap):
        return ap.bitcast(dt.float32r) if MM_DT == "fp32r" else ap

    nc = bass.Bass()
    xs = nc.dram_tensor("xs", [T, CIN, SLAB, XW], sdt, kind="ExternalInput")
    w3 = nc.dram_tensor("w3", [96, 768], sdt, kind="ExternalInput")
    ua = nc.dram_tensor("ua", [128, 768], sdt, kind="ExternalInput")
    ub = nc.dram_tensor("ub", [128, 256], sdt, kind="ExternalInput")
    ubs = nc.dram_tensor("ubs", [64, 256], sdt, kind="ExternalInput")
    vecs = nc.dram_tensor("vecs", [128, 8], dt.float32, kind="ExternalInput")
    y = nc.dram_tensor("y", [T, F, R * W2], dt.float32, kind="ExternalOutput")

    with tile.TileContext(nc) as tc:
        with (
            tc.tile_pool(name="const", bufs=1) as cpool,
            tc.tile_pool(name="state", bufs=1) as spool,
            tc.tile_pool(name="xp", bufs=2) as xpool,
            tc.tile_pool(name="ps", bufs=4, space="PSUM") as pspool,
            tc.tile_pool(name="epi", bufs=2) as epool,
        ):
            w3sb = cpool.tile([96, 768], sdt, tag="w3sb")
            uasb = cpool.tile([128, 768], sdt, tag="uasb")
            ubsb = cpool.tile([128, 256], sdt, tag="ubsb")
            ubssb = cpool.tile([64, 256], sdt, tag="ubssb")
            vsb = cpool.tile([128, 8], dt.float32, tag="vsb")
            nc.sync.dma_start(out=w3sb[:], in_=w3[:])
            nc.sync.dma_start(out=uasb[:], in_=ua[:])
            nc.sync.dma_start(out=ubsb[:], in_=ub[:])
            nc.sync.dma_start(out=ubssb[:], in_=ubs[:])
            nc.sync.dma_start(out=vsb[:], in_=vecs[:])

            h2 = [spool.tile([128, (RC + 2) * WP], sdt, name=f"h2_{i}",
                             tag=f"h2_{i}")
                  for i in range(2)]
            h3 = [spool.tile([128, (RC + 2) * WP3], sdt, name=f"h3_{i}",
                             tag=f"h3_{i}")
                  for i in range(2)]
            nc.vector.memset(h3[0][:], 0.0)
            nc.vector.memset(h3[1][:], 0.0)
            # c state lives at partitions 64:128 (f32) so the f*c stt can
            # run with equal SB base partitions (64,64)
            gcr = spool.tile([128, RC * W2], dt.float32, tag="gcr")
            nc.vector.memset(h2[0][:], 0.0)
            nc.vector.memset(h2[1][:], 0.0)
            nc.vector.memset(gcr[64:128, :], 0.0)

            def mm_group(psl, m, q0, nq, x3r, hpr, h3pr):
                """9 accumulating matmuls for nq out-rows into psl
                ([128, nq*64] psum slice). Order w3 -> ub -> ua: w3 needs
                no h at all and ub only the primary h copy, so the
                row-shifted duplicate (written last in the epilogue chain)
                is only needed by the final 3 matmuls."""
                psr = psl.rearrange("p (a b) -> p a b", b=W2)
                idx = 0
                for di in range(3):
                    d = di - 1
                    gcol = slice((di*2+m)*128, (di*2+m+1)*128)
                    nc.tensor.matmul(
                        psr[:],
                        lhsT=mm_ap(w3sb[0:96, gcol]),
                        rhs=mm_ap(x3r[0:96, q0:q0+nq, d+1:d+129:2]),
                        start=(idx == 0), stop=False)
                    idx += 1
                mcol = slice(m*128, (m+1)*128)
                # (kh2, dj=0): K=64 on the primary h copy
                nc.tensor.matmul(
                    psr[:],
                    lhsT=mm_ap(ubssb[0:64, mcol]),
                    rhs=mm_ap(hpr[0:64, q0+2:q0+nq+2, 1:65]),
                    start=False, stop=False)
                for di in range(3):
                    d = di - 1
                    gcol = slice((di*2+m)*128, (di*2+m+1)*128)
                    nc.tensor.matmul(
                        psr[:],
                        lhsT=mm_ap(uasb[0:128, gcol]),
                        rhs=mm_ap(hpr[0:128, q0:q0+nq, 1+d:65+d]),
                        start=False, stop=False)
                    idx += 1
                # (kh2, dj=-1)+(kh2, dj=+1) in one K=128 matmul on the
                # column-shifted duplicate pair (needs h3, written last)
                nc.tensor.matmul(
                    psr[:],
                    lhsT=mm_ap(ubsb[0:128, mcol]),
                    rhs=mm_ap(h3pr[0:128, q0+2:q0+nq+2, 2:66]),
                    start=False, stop=True)

            def epilogue(t, psA, psB, q0, nw, hcr, h3cr, own):
                """Gates/LSTM for nw (8 or 16) out-rows starting at q0.
                psA = [z_i; z_f], psB = [z_g; z_o], each [128, nw*64].
                Gate intermediates are bf16 (2x DVE rate); c stays f32."""
                n = nw * W2
                cs = slice(q0 * W2, q0 * W2 + n)
                if_t = epool.tile([128, n], sdt, tag=f"if{nw}")
                nc.scalar.activation(if_t[:], psA[:], AF.Relu,
                                     bias=vsb[0:128, 0:1], scale=0.2)
                g_t = epool.tile([64, n], sdt, tag=f"g{nw}")
                nc.scalar.activation(g_t[:], psB[0:64, :], AF.Tanh,
                                     bias=vsb[0:64, 1:2], scale=1.0)
                # gate products: f*c into the dead psA bank (SB bases
                # 64,64), i*g to SBUF (bases 0,0). The combining add reads
                # one SB + one PSUM input, exempt from both the SB-SB
                # equal-base rule (NCC_IBIR297) and the one-PSUM-input
                # rule (NCC_IBVF027).
                nc.vector.scalar_tensor_tensor(
                    psA[64:128, :], if_t[64:128, :], 1.0, gcr[64:128, cs],
                    ALU.min, ALU.mult)
                t2 = epool.tile([64, n], sdt, tag=f"t2{nw}")
                nc.vector.scalar_tensor_tensor(
                    t2[:], if_t[0:64, :], 1.0, g_t[:],
                    ALU.min, ALU.mult)
                nc.vector.tensor_add(gcr[64:128, cs], t2[:],
                                     psA[64:128, :])
                tc_t = epool.tile([64, n], sdt, tag=f"tc{nw}")
                nc.scalar.activation(tc_t[:], gcr[64:128, cs], AF.Tanh)
                o_t = epool.tile([64, n], sdt, tag=f"o{nw}")
                nc.scalar.activation(o_t[:], psB[64:128, :], AF.Relu,
                                     bias=vsb[64:128, 2:3], scale=0.2)
                hlo = hcr[0:64, q0+1:q0+nw+1, 1:65]
                nc.vector.scalar_tensor_tensor(
                    hlo,
                    o_t[:].rearrange("p (a b) -> p a b", b=W2), 1.0,
                    tc_t[:].rearrange("p (a b) -> p a b", b=W2),
                    ALU.min, ALU.mult)
                # h duplicate (row-shifted for the 2-tap ua matmuls) on the
                # ACT engine: gpsimd COPY measured ~2.8us, ACT ~1.2us
                nc.scalar.activation(hcr[64:128, q0:q0+nw, 1:65], hlo,
                                     AF.Copy)
                # column-shifted duplicates for the kh2 dj=+-1 pair matmul:
                # parts 0:64 at col base 3 (reads c-1), 64:128 at col base
                # 1 (reads c+1); same rows in both halves
                nc.vector.tensor_copy(
                    out=h3cr[0:64, q0+1:q0+nw+1, 3:67], in_=hlo)
                nc.gpsimd.tensor_copy(
                    out=h3cr[64:128, q0+1:q0+nw+1, 1:65], in_=hlo)
                if own:
                    yst = epool.tile([64, n], dt.float32, tag=f"y{nw}")
                    nc.gpsimd.tensor_scalar(
                        yst[:].rearrange("p (a b) -> p a b", b=W2), hlo,
                        vsb[0:64, 3:4], vsb[0:64, 4:5],
                        ALU.mult, ALU.add)
                    nc.sync.dma_start(out=y[t, :, q0*W2:q0*W2+n],
                                      in_=yst[:])

            for t in range(T):
                hc = h2[t % 2]
                hp = h2[(t + 1) % 2]
                hcr = hc[:].rearrange("p (q w) -> p q w", w=WP)
                hpr = hp[:].rearrange("p (q w) -> p q w", w=WP)
                h3cr = h3[t % 2][:].rearrange("p (q w) -> p q w", w=WP3)
                h3pr = h3[(t + 1) % 2][:].rearrange("p (q w) -> p q w",
                                                    w=WP3)
                # rows needed this step = 32 + (15 - t), rounded up to a
                # multiple of 4 (the finest chunk granularity)
                rows = min(RC, ((32 + (15 - t)) + 3) // 4 * 4)

                x3t = xpool.tile([96, RC * XW], sdt, tag="x3")
                x3r = x3t[:].rearrange("p (q w) -> p q w", w=XW)
                nc.sync.dma_start(out=x3r[0:32], in_=xs[t, :, 0:2*RC-1:2, :])
                nc.sync.dma_start(out=x3r[32:64], in_=xs[t, :, 1:2*RC:2, :])
                nc.sync.dma_start(out=x3r[64:96], in_=xs[t, :, 2:2*RC+1:2, :])

                # paired chunks: [128,1024] psum tiles = 2 banks each
                for k in range(rows // 16):
                    q0 = k * 2 * NQ
                    psA = pspool.tile([128, 1024], dt.float32,
                                      name=f"psA_{t}_{k}", tag="ps")
                    psB = pspool.tile([128, 1024], dt.float32,
                                      name=f"psB_{t}_{k}", tag="ps")
                    for half in range(2):
                        hs = slice(half * 512, half * 512 + 512)
                        mm_group(psA[:, hs], 0, q0 + half * NQ, NQ, x3r, hpr, h3pr)
                        mm_group(psB[:, hs], 1, q0 + half * NQ, NQ, x3r, hpr, h3pr)
                    epilogue(t, psA[:], psB[:], q0, 2 * NQ, hcr, h3cr, own=(k < 2))

                # redundant tail: an 8-row and/or a 4-row chunk, both
                # m-halves side by side in one 2-bank tile
                q0 = rows // 16 * 16
                while q0 < rows:
                    nq = min(8, rows - q0)
                    ps = pspool.tile([128, 1024], dt.float32,
                                     name=f"ps1_{t}_{q0}", tag="ps")
                    mm_group(ps[:, 0:nq*W2], 0, q0, nq, x3r, hpr, h3pr)
                    mm_group(ps[:, 512:512+nq*W2], 1, q0, nq, x3r, hpr, h3pr)
                    epilogue(t, ps[:, 0:nq*W2], ps[:, 512:512+nq*W2], q0,
                             nq, hcr, h3cr, own=False)
                    q0 += nq
    _split_multi_waits(nc, mybir)
    return nc


def _install_ntff_hook():
    """The image's antenv lacks axon_hooks; synthesize it and register the
    ctypes NTFF profile hook so trace=True works under axon."""
    import sys
    import types
    try:
        from antenv.axon_hooks import get_axon_ntff_profile_hook  # noqa
        return
    except ImportError:
        pass
    mod = types.ModuleType("antenv.axon_hooks")
    mod._hook = None

    def set_axon_ntff_profile_hook(h):
        mod._hook = h

    def get_axon_ntff_profile_hook():
        return mod._hook

    mod.set_axon_ntff_profile_hook = set_axon_ntff_profile_hook
    mod.get_axon_ntff_profile_hook = get_axon_ntff_profile_hook
    sys.modules["antenv.axon_hooks"] = mod
    import antenv
    antenv.axon_hooks = mod
    try:
        from trn_agent_boot.trn_boot import _ntff_profile_via_ctypes
        hook = _ntff_profile_via_ctypes("/opt/axon/libaxon_pjrt.so")
        if hook is not None:
            mod._hook = hook
    except Exception:
        pass


def _get_nc():
    key = (MM_DT,)
    if key not in _CACHE:
        _CACHE[key] = _build_nc()
    return _CACHE[key]


def kernel(x, W, U, b, gamma, beta, moving_mean, moving_var):
    from concourse.bass_utils import run_bass_kernel_spmd
    x = np.asarray(x, np.float32)
    W = np.asarray(W, np.float32)
    U = np.asarray(U, np.float32)
    b = np.asarray(b, np.float32)
    gamma = np.asarray(gamma, np.float32)
    beta = np.asarray(beta, np.float32)
    moving_mean = np.asarray(moving_mean, np.float32)
    moving_var = np.asarray(moving_var, np.float32)
    B = x.shape[0]

    in_maps = []
    for bidx in range(B):
        for half in range(2):
            in_maps.append(_prep_core_inputs(
                x, W, U, b, gamma, beta, moving_mean, moving_var, bidx, half))

    nc = _get_nc()
    trace = os.environ.get("BASS_KERNEL_TRACE") == "1"
    if trace:
        _install_ntff_hook()
    res = run_bass_kernel_spmd(nc, in_maps, core_ids=list(range(8)),
                               trace=trace)
    kernel._last_result = res

    out = np.zeros((B, T, H2, W2, F), np.float32)
    ci = 0
    for bidx in range(B):
        for half in range(2):
            yc = res.results[ci]["y"].reshape(T, F, R, W2)
            ci += 1
            yc = yc.transpose(0, 2, 3, 1)  # (T, R, W2, F)
            if half == 1:
                yc = yc[:, ::-1, :, :]
                out[bidx, :, 32:64] = yc
            else:
                out[bidx, :, 0:32] = yc
    return out


# revision 19
# speedup vs baseline: 1.2753x; 1.2753x over previous
"""ConvLSTM block Trainium2 kernel (8 NeuronCores).

Sharding: 8 cores = 4 batches x 2 H-halves. Bottom-half cores process their
slab vertically flipped (with kh-flipped conv kernels) so one SPMD program
serves all cores.

Cross-core traffic is ELIMINATED by redundant computation: each core
computes 48 rows/step for steps 1-7, 40 rows/step for steps 8-15, and 32
rows at step 16. The valid region shrinks by one row per step from 48, so
rows 0..31 stay valid through all 16 steps with no exchange at all (a
measured 2-rank AllReduce costs ~37us and stalls the PE - redundant matmuls
are cheaper).

Per step, chunks of 8 output rows are processed in PAIRS sharing a 2-bank
[128,1024] PSUM tile per m-half, so the epilogue runs [*,1024]-wide ops:
ACT: merged hard-sigmoid(i,f) on 128 partitions, tanh g, hard-sigmoid o,
tanh c; DVE: one stt for both gate products, h-store; GpSimd: c-combine add,
h duplicate copy, fused BN (tensor_scalar mult+add).
"""
import os
import numpy as np

T, H2, W2, F, CIN = 16, 64, 64, 64, 32
R = 32            # rows owned per core
RC = 48           # max rows computed per step (redundant tail shrinks)
NQ = 8            # rows per chunk (512 px = 1 PSUM bank per 128ch tile)
WP, XW = 66, 130
SLAB = 2 * RC + 1
MM_DT = os.environ.get("CONV_LSTM_MM_DT", "bf16")  # bf16 | fp32 | fp32r

_CACHE = {}


def _storage_np_dtype():
    import ml_dtypes
    return ml_dtypes.bfloat16 if MM_DT == "bf16" else np.float32


def _prep_core_inputs(x, W, U, b, gamma, beta, moving_mean, moving_var,
                      bidx, half):
    sdt = _storage_np_dtype()
    flip = (half == 1)

    # x slab [T, CIN, SLAB, XW]; XLA SAME (stride2,k3,even) pads bottom/right
    # only: out row r reads input rows 2r..2r+2 (row/col 128 = zero pad).
    xs = np.zeros((T, CIN, SLAB, XW), np.float32)
    xc = np.ascontiguousarray(x[bidx].transpose(0, 3, 1, 2))  # (T,CIN,128,128)
    if not flip:
        xs[:, :, 0:SLAB, 0:128] = xc[:, :, 0:SLAB, :]
    else:
        # slab[s] = x_global[128 - s]; s=0 is the zero pad row
        xs[:, :, 1:SLAB, 0:128] = xc[:, :, 128 - SLAB + 1:][:, :, ::-1, :]

    Wk = W[::-1].copy() if flip else W
    Uk = U[::-1].copy() if flip else U

    w3 = np.zeros((96, 768), np.float32)
    ua = np.zeros((128, 768), np.float32)
    ub = np.zeros((128, 768), np.float32)
    for di in range(3):
        for m in range(2):
            g = di * 2 + m
            cols = slice(g * 128, (g + 1) * 128)
            mc = slice(m * 128, (m + 1) * 128)
            for j in range(3):
                w3[32*j:32*j+32, cols] = Wk[j, di, :, mc]
            ua[0:64, cols] = Uk[0, di, :, mc]
            ua[64:128, cols] = Uk[1, di, :, mc]
            ub[0:64, cols] = Uk[2, di, :, mc]

    eps = 1e-3
    scale = (gamma / np.sqrt(moving_var + eps)).astype(np.float32)
    beta2 = (beta - moving_mean * scale).astype(np.float32)
    vecs = np.zeros((128, 8), np.float32)
    vecs[:, 0] = 0.2 * b[0:128] + 0.5
    vecs[0:64, 1] = b[128:192]
    vecs[64:128, 2] = 0.2 * b[192:256] + 0.5
    vecs[0:64, 3] = scale
    vecs[0:64, 4] = beta2
    return {
        "xs": np.ascontiguousarray(xs.astype(sdt)),
        "w3": np.ascontiguousarray(w3.astype(sdt)),
        "ua": np.ascontiguousarray(ua.astype(sdt)),
        "ub": np.ascontiguousarray(ub.astype(sdt)),
        "vecs": vecs,
    }


def _patch_tile_drain():
    """This walrus build encodes at most ONE sync wait per CTRL instruction;
    split the Tile exit drain's waits across SP nops."""
    import bass_rust
    import concourse.tile as tile
    from concourse.vector_clock import ScopedClock
    if getattr(tile.TileContext, "_drain_patched", False):
        return

    def patched(self, tick_clock, wait_clock):
        drain_inst = self.nc.sync.drain()
        wait_clock.add_sem_waits(
            drain_inst.ins, ScopedClock({None: tick_clock.global_clock}))
        si = drain_inst.ins.sync_info
        waits = list(si.on_wait) if si is not None else []
        if len(waits) > 1:
            si.on_wait = waits[:1]
            for w in waits[1:]:
                nop = self.nc.sync.nop()
                nsi = nop.ins.sync_info
                if nsi is None:
                    nop.ins.sync_info = bass_rust.SyncInfo(
                        on_wait=[w], on_update=[])
                else:
                    nsi.on_wait = [w]
        self.nc.all_engine_barrier()
        assert self.sems is not None
        popped = self.nc._tile_sem_poison_stack.pop()
        assert popped is self._sem_poison
        self.nc.clear_and_free_semaphores(list(self.sems.allocated().values()))
        self.nc.all_engine_barrier()

    tile.TileContext._drain_and_barrier = patched
    tile.TileContext._drain_patched = True


def _split_multi_waits(nc, mybir):
    """This walrus build encodes at most one sync wait per instruction;
    move excess waits onto single-wait nops inserted just before."""
    ctr = 0
    for bb in nc.main_func.blocks:
        insts = bb.instructions
        out = []
        changed = False
        for inst in insts:
            si = inst.sync_info
            waits = list(si.on_wait) if si is not None else []
            if len(waits) > 1:
                changed = True
                for w in waits[:-1]:
                    ctr += 1
                    out.append(mybir.InstNoOp(
                        name=f"wsplit-{ctr}",
                        engine=inst.engine,
                        sync_info=mybir.SyncInfo(on_wait=[w], on_update=[]),
                        bass_nofuse=True))
                si.on_wait = [waits[-1]]
            out.append(inst)
        if changed:
            bb.instructions = out
    return nc


def _build_nc():
    import concourse.bass as bass
    import concourse.mybir as mybir
    import concourse.tile as tile
    _patch_tile_drain()
    dt = mybir.dt
    sdt = dt.bfloat16 if MM_DT == "bf16" else dt.float32
    AF = mybir.ActivationFunctionType
    ALU = mybir.AluOpType

    def mm_ap(ap):
        return ap.bitcast(dt.float32r) if MM_DT == "fp32r" else ap

    nc = bass.Bass()
    xs = nc.dram_tensor("xs", [T, CIN, SLAB, XW], sdt, kind="ExternalInput")
    w3 = nc.dram_tensor("w3", [96, 768], sdt, kind="ExternalInput")
    ua = nc.dram_tensor("ua", [128, 768], sdt, kind="ExternalInput")
    ub = nc.dram_tensor("ub", [128, 768], sdt, kind="ExternalInput")
    vecs = nc.dram_tensor("vecs", [128, 8], dt.float32, kind="ExternalInput")
    y = nc.dram_tensor("y", [T, F, R * W2], dt.float32, kind="ExternalOutput")

    with tile.TileContext(nc) as tc:
        with (
            tc.tile_pool(name="const", bufs=1) as cpool,
            tc.tile_pool(name="state", bufs=1) as spool,
            tc.tile_pool(name="xp", bufs=2) as xpool,
            tc.tile_pool(name="ps", bufs=4, space="PSUM") as pspool,
            tc.tile_pool(name="epi", bufs=2) as epool,
        ):
            w3sb = cpool.tile([96, 768], sdt, tag="w3sb")
            uasb = cpool.tile([128, 768], sdt, tag="uasb")
            ubsb = cpool.tile([128, 768], sdt, tag="ubsb")
            vsb = cpool.tile([128, 8], dt.float32, tag="vsb")
            nc.sync.dma_start(out=w3sb[:], in_=w3[:])
            nc.sync.dma_start(out=uasb[:], in_=ua[:])
            nc.sync.dma_start(out=ubsb[:], in_=ub[:])
            nc.sync.dma_start(out=vsb[:], in_=vecs[:])

            h2 = [spool.tile([128, (RC + 2) * WP], sdt, name=f"h2_{i}",
                             tag=f"h2_{i}")
                  for i in range(2)]
            # c state lives at partitions 64:128 (f32) so the f*c stt can
            # run with equal SB base partitions (64,64)
            gcr = spool.tile([128, RC * W2], dt.float32, tag="gcr")
            nc.vector.memset(h2[0][:], 0.0)
            nc.vector.memset(h2[1][:], 0.0)
            nc.vector.memset(gcr[64:128, :], 0.0)

            def mm_group(psl, m, q0, nq, x3r, hpr):
                """9 accumulating matmuls for nq out-rows into psl
                ([128, nq*64] psum slice). Order w3 -> ub -> ua: w3 needs
                no h at all and ub only the primary h copy, so the
                row-shifted duplicate (written last in the epilogue chain)
                is only needed by the final 3 matmuls."""
                psr = psl.rearrange("p (a b) -> p a b", b=W2)
                idx = 0
                for di in range(3):
                    d = di - 1
                    gcol = slice((di*2+m)*128, (di*2+m+1)*128)
                    nc.tensor.matmul(
                        psr[:],
                        lhsT=mm_ap(w3sb[0:96, gcol]),
                        rhs=mm_ap(x3r[0:96, q0:q0+nq, d+1:d+129:2]),
                        start=(idx == 0), stop=False)
                    idx += 1
                for di in range(3):
                    d = di - 1
                    gcol = slice((di*2+m)*128, (di*2+m+1)*128)
                    nc.tensor.matmul(
                        psr[:],
                        lhsT=mm_ap(ubsb[0:128, gcol]),
                        rhs=mm_ap(hpr[0:128, q0+2:q0+nq+2, 1+d:65+d]),
                        start=False, stop=False)
                    idx += 1
                for di in range(3):
                    d = di - 1
                    gcol = slice((di*2+m)*128, (di*2+m+1)*128)
                    nc.tensor.matmul(
                        psr[:],
                        lhsT=mm_ap(uasb[0:128, gcol]),
                        rhs=mm_ap(hpr[0:128, q0:q0+nq, 1+d:65+d]),
                        start=False, stop=(idx == 8))
                    idx += 1

            def epilogue(t, psA, psB, q0, nw, hcr, own):
                """Gates/LSTM for nw (8 or 16) out-rows starting at q0.
                psA = [z_i; z_f], psB = [z_g; z_o], each [128, nw*64].
                Gate intermediates are bf16 (2x DVE rate); c stays f32."""
                n = nw * W2
                cs = slice(q0 * W2, q0 * W2 + n)
                if_t = epool.tile([128, n], sdt, tag=f"if{nw}")
                nc.scalar.activation(if_t[:], psA[:], AF.Relu,
                                     bias=vsb[0:128, 0:1], scale=0.2)
                g_t = epool.tile([64, n], sdt, tag=f"g{nw}")
                nc.scalar.activation(g_t[:], psB[0:64, :], AF.Tanh,
                                     bias=vsb[0:64, 1:2], scale=1.0)
                # gate products: f*c into the dead psA bank (SB bases
                # 64,64), i*g to SBUF (bases 0,0). The combining add reads
                # one SB + one PSUM input, exempt from both the SB-SB
                # equal-base rule (NCC_IBIR297) and the one-PSUM-input
                # rule (NCC_IBVF027).
                nc.vector.scalar_tensor_tensor(
                    psA[64:128, :], if_t[64:128, :], 1.0, gcr[64:128, cs],
                    ALU.min, ALU.mult)
                t2 = epool.tile([64, n], sdt, tag=f"t2{nw}")
                nc.vector.scalar_tensor_tensor(
                    t2[:], if_t[0:64, :], 1.0, g_t[:],
                    ALU.min, ALU.mult)
                nc.vector.tensor_add(gcr[64:128, cs], t2[:],
                                     psA[64:128, :])
                tc_t = epool.tile([64, n], sdt, tag=f"tc{nw}")
                nc.scalar.activation(tc_t[:], gcr[64:128, cs], AF.Tanh)
                o_t = epool.tile([64, n], sdt, tag=f"o{nw}")
                nc.scalar.activation(o_t[:], psB[64:128, :], AF.Relu,
                                     bias=vsb[64:128, 2:3], scale=0.2)
                hlo = hcr[0:64, q0+1:q0+nw+1, 1:65]
                nc.vector.scalar_tensor_tensor(
                    hlo,
                    o_t[:].rearrange("p (a b) -> p a b", b=W2), 1.0,
                    tc_t[:].rearrange("p (a b) -> p a b", b=W2),
                    ALU.min, ALU.mult)
                # h duplicate (row-shifted for the 2-tap ua matmuls) on the
                # ACT engine: gpsimd COPY measured ~2.8us, ACT ~1.2us
                nc.scalar.activation(hcr[64:128, q0:q0+nw, 1:65], hlo,
                                     AF.Copy)
                if own:
                    yst = epool.tile([64, n], dt.float32, tag=f"y{nw}")
                    nc.gpsimd.tensor_scalar(
                        yst[:].rearrange("p (a b) -> p a b", b=W2), hlo,
                        vsb[0:64, 3:4], vsb[0:64, 4:5],
                        ALU.mult, ALU.add)
                    nc.sync.dma_start(out=y[t, :, q0*W2:q0*W2+n],
                                      in_=yst[:])

            for t in range(T):
                hc = h2[t % 2]
                hp = h2[(t + 1) % 2]
                hcr = hc[:].rearrange("p (q w) -> p q w", w=WP)
                hpr = hp[:].rearrange("p (q w) -> p q w", w=WP)
                # rows needed this step = 32 + (15 - t), rounded up to a
                # multiple of 4 (the finest chunk granularity)
                rows = min(RC, ((32 + (15 - t)) + 3) // 4 * 4)

                x3t = xpool.tile([96, RC * XW], sdt, tag="x3")
                x3r = x3t[:].rearrange("p (q w) -> p q w", w=XW)
                nc.sync.dma_start(out=x3r[0:32], in_=xs[t, :, 0:2*RC-1:2, :])
                nc.sync.dma_start(out=x3r[32:64], in_=xs[t, :, 1:2*RC:2, :])
                nc.sync.dma_start(out=x3r[64:96], in_=xs[t, :, 2:2*RC+1:2, :])

                # paired chunks: [128,1024] psum tiles = 2 banks each
                for k in range(rows // 16):
                    q0 = k * 2 * NQ
                    psA = pspool.tile([128, 1024], dt.float32,
                                      name=f"psA_{t}_{k}", tag="ps")
                    psB = pspool.tile([128, 1024], dt.float32,
                                      name=f"psB_{t}_{k}", tag="ps")
                    for half in range(2):
                        hs = slice(half * 512, half * 512 + 512)
                        mm_group(psA[:, hs], 0, q0 + half * NQ, NQ, x3r, hpr)
                        mm_group(psB[:, hs], 1, q0 + half * NQ, NQ, x3r, hpr)
                    epilogue(t, psA[:], psB[:], q0, 2 * NQ, hcr, own=(k < 2))

                # redundant tail: an 8-row and/or a 4-row chunk, both
                # m-halves side by side in one 2-bank tile
                q0 = rows // 16 * 16
                while q0 < rows:
                    nq = min(8, rows - q0)
                    ps = pspool.tile([128, 1024], dt.float32,
                                     name=f"ps1_{t}_{q0}", tag="ps")
                    mm_group(ps[:, 0:nq*W2], 0, q0, nq, x3r, hpr)
                    mm_group(ps[:, 512:512+nq*W2], 1, q0, nq, x3r, hpr)
                    epilogue(t, ps[:, 0:nq*W2], ps[:, 512:512+nq*W2], q0,
                             nq, hcr, own=False)
                    q0 += nq
    _split_multi_waits(nc, mybir)
    return nc


def _install_ntff_hook():
    """The image's antenv lacks axon_hooks; synthesize it and register the
    ctypes NTFF profile hook so trace=True works under axon."""
    import sys
    import types
    try:
        from antenv.axon_hooks import get_axon_ntff_profile_hook  # noqa
        return
    except ImportError:
        pass
    mod = types.ModuleType("antenv.axon_hooks")
    mod._hook = None

    def set_axon_ntff_profile_hook(h):
        mod._hook = h

    def get_axon_ntff_profile_hook():
        return mod._hook

    mod.set_axon_ntff_profile_hook = set_axon_ntff_profile_hook
    mod.get_axon_ntff_profile_hook = get_axon_ntff_profile_hook
    sys.modules["antenv.axon_hooks"] = mod
    import antenv
    antenv.axon_hooks = mod
    try:
        from trn_agent_boot.trn_boot import _ntff_profile_via_ctypes
        hook = _ntff_profile_via_ctypes("/opt/axon/libaxon_pjrt.so")
        if hook is not None:
            mod._hook = hook
    except Exception:
        pass


def _get_nc():
    key = (MM_DT,)
    if key not in _CACHE:
        _CACHE[key] = _build_nc()
    return _CACHE[key]


def kernel(x, W, U, b, gamma, beta, moving_mean, moving_var):
    from concourse.bass_utils import run_bass_kernel_spmd
    x = np.asarray(x, np.float32)
    W = np.asarray(W, np.float32)
    U = np.asarray(U, np.float32)
    b = np.asarray(b, np.float32)
    gamma = np.asarray(gamma, np.float32)
    beta = np.asarray(beta, np.float32)
    moving_mean = np.asarray(moving_mean, np.float32)
    moving_var = np.asarray(moving_var, np.float32)
    B = x.shape[0]

    in_maps = []
    for bidx in range(B):
        for half in range(2):
            in_maps.append(_prep_core_inputs(
                x, W, U, b, gamma, beta, moving_mean, moving_var, bidx, half))

    nc = _get_nc()
    trace = os.environ.get("BASS_KERNEL_TRACE") == "1"
    if trace:
        _install_ntff_hook()
    res = run_bass_kernel_spmd(nc, in_maps, core_ids=list(range(8)),
                               trace=trace)
    kernel._last_result = res

    out = np.zeros((B, T, H2, W2, F), np.float32)
    ci = 0
    for bidx in range(B):
        for half in range(2):
            yc = res.results[ci]["y"].reshape(T, F, R, W2)
            ci += 1
            yc = yc.transpose(0, 2, 3, 1)  # (T, R, W2, F)
            if half == 1:
                yc = yc[:, ::-1, :, :]
                out[bidx, :, 32:64] = yc
            else:
                out[bidx, :, 0:32] = yc
    return out
